# revision 6
# baseline (speedup 1.0000x reference)
"""GNN sparse-attention message passing on 8 Trainium2 NeuronCores (v2).

The axon tunnel (~47MB/s up, ~30MB/s down, shared serial channel) dominates
end-to-end time, so the kernel minimizes host<->device bytes and overlaps
host work with the upload:
- k, v, q ship as ONE int8 table [N, 390] SHARDED (N/8 rows per core):
  row = [k int8 x128 | v int8 x128 | kscale f16 | vscale f16 | q int8 x128 |
  qscale f16], every tensor per-row-scaled. The kv half is all-gathered on
  device over NeuronLink into a full DRAM table; the q half stays local
  (edges are sharded by destination node).
- Edge indices ship as uint16 src ids laid out in tile slots, plus a small
  per-group metadata table (out row, group base, cumulative edge counts).
  The destination one-hot matrices are reconstructed ON DEVICE from the
  cumulative counts (edges are dst-sorted), so no per-edge dst bytes ship.
- Per-edge q rows are NOT gathered row-by-row: each group's dst window is
  <=128 consecutive nodes, so the q window loads with one 128-descriptor
  gather per GROUP and a one-hot matmul (sel[n,e]) routes window rows to
  edge slots. This halves SWDGE descriptor generation (the device-side
  bottleneck, ~150ns/row) vs gathering q per edge.
- Output is quantized on device to int8 with a per-row fp16 scale; all
  scatter trash rows collapse onto one extra row so the download slab is
  [N/8+1, 130] per core. The host prefetches the download and dequantizes
  shard-by-shard as bytes arrive.

Compute (edge parallelism, sharded by destination node):
- Sort edges by dst (uint16 radix argsort); core c owns dst range
  [c*6250, (c+1)*6250).
- Per core, pack edges into groups of G tiles x 128 edges; each group's dst
  nodes lie in a window of <=128 consecutive node ids. cum[n] = number of
  group edges with dst < base+n gives both one-hot masks by comparison
  against an edge-slot iota: st[e,n] = (cum[n] <= e < cum[n+1]).
- Per tile: gather kv rows per edge via indirect DMA, widen int8->f32;
  qe = sel^T @ qwin (one-hot matmul); score = exp(clip(ks*qs*sum k*q / 4));
  msg = v * score * vs; one-hot matmul accumulates [wV | Z] in PSUM.
- Divide, row-quantize, indirect-scatter window rows to the output slab.
"""
import math

import numpy as np

import concourse.bass as bass
import concourse.tile as tile
from concourse import bacc, mybir

N = 50000
E = 800000
HID = 128
HEADS = 8
HD = 16
NCORES = 8
SH = N // NCORES          # nodes per core (fixed bounds)
G = 12                    # tiles per group
P = 128
GP = G * P
KVW = 2 * HID + 4         # kv row: 128 k + 128 v + f16 k-scale + f16 v-scale
QW = HID + 2              # q row: 128 q + f16 q-scale
TW = KVW + QW             # one merged upload row per node
CLIP_LO = float(np.exp(-5.0))
CLIP_HI = float(np.exp(5.0))

_cache = {}

_C_SRC = r"""
#include <stdint.h>
#include <math.h>

static inline uint16_t f32_to_f16(float f) {
    uint32_t x; __builtin_memcpy(&x, &f, 4);
    uint32_t r = x + 0x00001000u;              /* round mantissa (carries ok) */
    uint32_t sign = (r >> 16) & 0x8000u;
    int32_t exp = (int32_t)((r >> 23) & 0xff) - 127 + 15;
    uint32_t man = r & 0x7fffffu;
    if (exp <= 0) return (uint16_t)sign;              /* flush tiny to 0 */
    if (exp >= 31) return (uint16_t)(sign | 0x7bffu); /* clamp to max finite */
    return (uint16_t)(sign | ((uint32_t)exp << 10) | (man >> 13));
}

static inline float f16_to_f32(uint16_t h) {
    uint32_t sign = ((uint32_t)h & 0x8000u) << 16;
    uint32_t exp = (h >> 10) & 0x1fu;
    uint32_t man = h & 0x3ffu;
    uint32_t x;
    if (exp == 0) {
        if (!man) x = sign;
        else {
            int sh = 0;
            while (!(man & 0x400u)) { man <<= 1; sh++; }
            man &= 0x3ffu;
            x = sign | ((uint32_t)(113 - sh) << 23) | (man << 13);
        }
    } else if (exp == 31) {
        x = sign | 0x7f800000u | (man << 13);
    } else {
        x = sign | ((exp - 15 + 127) << 23) | (man << 13);
    }
    float f; __builtin_memcpy(&f, &x, 4);
    return f;
}

/* per-row absmax int8 quant; writes values + f16 scale into the packed table */
void quant_rows(const float* restrict x, int64_t nrows, int64_t w,
                int8_t* restrict tab, int64_t tab_stride,
                int64_t val_off, int64_t scale_off) {
    for (int64_t i = 0; i < nrows; i++) {
        const float* row = x + i * w;
        float m = 0.0f;
        for (int64_t j = 0; j < w; j++) {
            float a = fabsf(row[j]);
            m = a > m ? a : m;
        }
        float scale = m * (1.0f / 127.0f) + 1e-30f;
        float inv = 1.0f / scale;
        int8_t* dst = tab + i * tab_stride + val_off;
        for (int64_t j = 0; j < w; j++) {
            dst[j] = (int8_t)(int32_t)rintf(row[j] * inv);
        }
        uint16_t hb = f32_to_f16(scale);
        __builtin_memcpy(tab + i * tab_stride + scale_off, &hb, 2);
    }
}

/* int8 row * f16 row-scale -> f32 */
void dequant_rows(const int8_t* restrict xs, int64_t nrows, int64_t w,
                  int64_t stride, float* restrict out) {
    for (int64_t i = 0; i < nrows; i++) {
        const int8_t* row = xs + i * stride;
        uint16_t hb; __builtin_memcpy(&hb, row + w, 2);
        float s = f16_to_f32(hb);
        float* o = out + i * w;
        for (int64_t j = 0; j < w; j++) o[j] = (float)row[j] * s;
    }
}
"""


def _get_clib():
    """Compile the streaming quant/dequant helper once; None -> numpy path."""
    if "clib" in _cache:
        return _cache["clib"]
    lib = None
    try:
        import ctypes, subprocess, tempfile, os
        d = tempfile.mkdtemp(prefix="gnnq_")
        src = os.path.join(d, "q.c")
        so = os.path.join(d, "q.so")
        with open(src, "w") as f:
            f.write(_C_SRC)
        for flags in (["-O3", "-march=native", "-ffast-math", "-fno-math-errno"],
                      ["-O2"]):
            try:
                subprocess.run(["cc", *flags, "-shared", "-fPIC", src, "-o", so],
                               check=True, capture_output=True, timeout=60)
                lib = ctypes.CDLL(so)
                lib.quant_rows.argtypes = [
                    ctypes.c_void_p, ctypes.c_int64, ctypes.c_int64,
                    ctypes.c_void_p, ctypes.c_int64, ctypes.c_int64,
                    ctypes.c_int64]
                lib.dequant_rows.argtypes = [
                    ctypes.c_void_p, ctypes.c_int64, ctypes.c_int64,
                    ctypes.c_int64, ctypes.c_void_p]
                break
            except Exception:
                lib = None
    except Exception:
        lib = None
    _cache["clib"] = lib
    return lib


def _pack(e_src, e_dst):
    """Sort edges by dst, shard across fixed core ranges, pack into groups.

    Vectorized: python loops only over cores x groups (~500 iterations).
    Produces per core:
      srcs  [P, Gmax*G] u16 — src node id per edge slot (slot = t*128+p)
      meta4 [P, 4*Gmax] u16 — per group g cols 4g..4g+3 =
            [out row (or SH for trash), group base, cum[p], cum[p+1]]
      cumf  [Gmax, 129] u16 — the same cumulative counts, free-axis layout
    """
    order = np.argsort(e_dst.astype(np.uint16), kind="stable")
    s = e_src.astype(np.uint16)[order]
    d = e_dst.astype(np.int32)[order]
    deg = np.bincount(d, minlength=N)
    cum = np.concatenate([[0], np.cumsum(deg)])  # cum[n] = edges with dst < n

    # greedy group boundaries per core: window <=P nodes, <=G*P edges
    core_groups = []
    for c in range(NCORES):
        n0, n1 = c * SH, (c + 1) * SH
        bases = []
        ni = n0
        while ni < n1:
            bases.append(ni)
            cap_node = min(ni + P, n1)
            cap_edge = int(np.searchsorted(cum, cum[ni] + GP, side="right")) - 1
            ni = max(ni + 1, min(cap_node, cap_edge))
        core_groups.append(bases)
    Gmax = max(len(b) for b in core_groups)

    per_core = []
    r128 = np.arange(P)
    r129 = np.arange(P + 1)
    for c in range(NCORES):
        n0, n1 = c * SH, (c + 1) * SH
        bases = np.asarray(core_groups[c], np.int64)
        ng = len(bases)
        nxt = np.concatenate([bases[1:], [n1]])
        e0s, e1s = cum[bases], cum[nxt]          # edge ranges per group
        ne = e1s - e0s
        assert int(ne.max(initial=0)) <= GP, "group edge overflow"

        # src id for every edge slot of this core in one shot
        srcs = np.zeros((Gmax, GP), np.uint16)
        ce0, ce1 = cum[n0], cum[n1]
        slot = np.repeat(np.arange(ng) * GP - (e0s - ce0), ne) \
            + np.arange(ce1 - ce0)
        srcs.reshape(-1)[slot] = s[ce0:ce1]

        # group-local cumulative edge counts over the 128-node window
        idx = np.minimum(bases[:, None] + r129[None, :], nxt[:, None])
        cumg = (cum[idx] - e0s[:, None]).astype(np.uint16)      # [ng, 129]
        cumf = np.zeros((Gmax, P + 1), np.uint16)
        cumf[:ng] = cumg

        span = np.minimum(nxt - bases, P)                       # [ng]
        rows = (bases[:, None] - n0) + r128[None, :]            # [ng, P]
        m4 = np.zeros((Gmax, 4, P), np.uint16)
        m4[:, 0, :] = SH                                        # trash row
        m4[:ng, 0, :] = np.where(r128[None, :] < span[:, None], rows, SH)
        m4[:ng, 1, :] = (bases[:, None] - n0).astype(np.uint16)
        m4[:ng, 2, :] = cumg[:, :P]
        m4[:ng, 3, :] = cumg[:, 1:]

        per_core.append({
            "srcs": np.ascontiguousarray(
                srcs.reshape(Gmax, G, P).transpose(2, 0, 1)).reshape(P, Gmax * G),
            "meta4": np.ascontiguousarray(
                m4.transpose(2, 0, 1)).reshape(P, 4 * Gmax),
            "cumf": cumf,
        })
    return per_core, Gmax


def _build(Gmax):
    nc = bacc.Bacc(None, target_bir_lowering=False, num_devices=NCORES)
    f32 = mybir.dt.float32
    f16 = mybir.dt.float16
    i32 = mybir.dt.int32
    i8 = mybir.dt.int8
    u16 = mybir.dt.uint16
    tab = nc.declare_dram_parameter("tab", [SH, TW], i8, isOutput=False)
    srcs = nc.declare_dram_parameter("srcs", [P, Gmax * G], u16, isOutput=False)
    meta4 = nc.declare_dram_parameter("meta4", [P, 4 * Gmax], u16, isOutput=False)
    cumf = nc.declare_dram_parameter("cumf", [Gmax, P + 1], u16, isOutput=False)
    xout = nc.declare_dram_parameter("xout", [SH + 1, QW], i8, isOutput=True)

    # bounce buffers for the all-gather (collectives can't touch I/O tensors)
    agkv = nc.dram_tensor("agkv", [SH, KVW], i8)
    kvfull = nc.dram_tensor("kvfull", [N, KVW], i8)
    qfull = nc.dram_tensor("qfull", [SH, QW], i8)

    with tile.TileContext(nc) as tc:
        with tc.tile_pool(name="const", bufs=1) as cp, \
             tc.tile_pool(name="sbuf", bufs=4) as sb, \
             tc.tile_pool(name="meta", bufs=3) as mp, \
             tc.tile_pool(name="psum", bufs=2, space="PSUM") as ps:
            nc.sync.dma_start(out=agkv[:], in_=tab[:, :KVW])
            nc.sync.dma_start(out=qfull[:], in_=tab[:, KVW:])
            nc.gpsimd.collective_compute(
                "AllGather", mybir.AluOpType.bypass,
                replica_groups=[list(range(NCORES))],
                ins=[agkv[:].opt()], outs=[kvfull[:].opt()])

            # constants: edge-slot iota along free axis, partition iota
            eii = cp.tile([P, GP], i32)
            nc.gpsimd.iota(eii[:], pattern=[[1, GP]], base=0, channel_multiplier=0)
            eiota = cp.tile([P, GP], f32)
            nc.vector.tensor_copy(out=eiota[:], in_=eii[:])
            pii = cp.tile([P, 1], i32)
            nc.gpsimd.iota(pii[:], pattern=[[1, 1]], base=0, channel_multiplier=1)
            piota = cp.tile([P, 1], f32)
            nc.vector.tensor_copy(out=piota[:], in_=pii[:])
            ones1 = cp.tile([1, P], f32)
            nc.vector.memset(ones1[:], 1.0)

            for g in range(Gmax):
                srcs_sb = mp.tile([P, G], i32, tag="srcs")
                nc.gpsimd.dma_start(out=srcs_sb[:], in_=srcs[:, g * G:(g + 1) * G])
                m4 = mp.tile([P, 4], i32, tag="m4")
                nc.gpsimd.dma_start(out=m4[:], in_=meta4[:, 4 * g:4 * g + 4])
                cumf_i = mp.tile([1, P + 1], i32, tag="cumfi")
                nc.gpsimd.dma_start(out=cumf_i[:], in_=cumf[g:g + 1, :])
                cumf_f = mp.tile([1, P + 1], f32, tag="cumff")
                nc.vector.tensor_copy(out=cumf_f[:], in_=cumf_i[:])

                # broadcast cum over partitions: [1,129] -> PSUM [128,129]
                cumbc = ps.tile([P, P + 1], f32, space="PSUM", tag="cumbc")
                nc.tensor.matmul(out=cumbc[:], lhsT=ones1[:], rhs=cumf_f[:],
                                 start=True, stop=True)

                # q window rows: qoff = min(gbase + p, SH-1)
                qoff = mp.tile([P, 1], i32, tag="qoff")
                nc.vector.tensor_tensor(out=qoff[:], in0=m4[:, 1:2], in1=pii[:],
                                        op=mybir.AluOpType.add)
                nc.vector.tensor_scalar(out=qoff[:], in0=qoff[:], scalar1=SH - 1,
                                        scalar2=None, op0=mybir.AluOpType.min)
                qt8 = mp.tile([P, QW], i8, tag="qt8")
                nc.gpsimd.indirect_dma_start(
                    out=qt8[:], out_offset=None, in_=qfull[:],
                    in_offset=bass.IndirectOffsetOnAxis(ap=qoff[:, 0:1], axis=0))
                qwf = mp.tile([P, HID + 1], f32, tag="qwf")
                nc.vector.tensor_copy(out=qwf[:, :HID], in_=qt8[:, :HID])
                nc.vector.tensor_copy(
                    out=qwf[:, HID:HID + 1],
                    in_=qt8[:, HID:HID + 2].bitcast(f16))

                # sel[n, e] = (cum[n] <= e) & (e < cum[n+1]) over the whole group
                cumlh = mp.tile([P, 2], f32, tag="cumlh")
                nc.vector.tensor_copy(out=cumlh[:], in_=m4[:, 2:4])
                sel = mp.tile([P, GP], f32, tag="sel")
                nc.vector.tensor_tensor(
                    out=sel[:], in0=eiota[:],
                    in1=cumlh[:, 0:1].to_broadcast([P, GP]),
                    op=mybir.AluOpType.is_ge)
                sellt = mp.tile([P, GP], f32, tag="sellt")
                nc.vector.tensor_tensor(
                    out=sellt[:], in0=eiota[:],
                    in1=cumlh[:, 1:2].to_broadcast([P, GP]),
                    op=mybir.AluOpType.is_lt)
                nc.vector.tensor_tensor(out=sel[:], in0=sel[:], in1=sellt[:],
                                        op=mybir.AluOpType.mult)

                acc = ps.tile([P, HID + HEADS], f32, space="PSUM", tag="acc")
                for t in range(G):
                    kvt8 = sb.tile([P, KVW], i8, tag="kvt8")
                    nc.gpsimd.indirect_dma_start(
                        out=kvt8[:], out_offset=None, in_=kvfull[:],
                        in_offset=bass.IndirectOffsetOnAxis(
                            ap=srcs_sb[:, t:t + 1], axis=0))
                    kvf = sb.tile([P, 2 * HID], f32, tag="kvf")
                    nc.vector.tensor_copy(out=kvf[:], in_=kvt8[:, :2 * HID])
                    ssc = sb.tile([P, 2], f32, tag="ssc")
                    nc.vector.tensor_copy(
                        out=ssc[:, 0:1],
                        in_=kvt8[:, 2 * HID:2 * HID + 2].bitcast(f16))
                    nc.vector.tensor_copy(
                        out=ssc[:, 1:2],
                        in_=kvt8[:, 2 * HID + 2:2 * HID + 4].bitcast(f16))

                    # per-edge q row via one-hot matmul
                    qe = ps.tile([P, HID + 1], f32, space="PSUM", tag="qe")
                    nc.tensor.matmul(out=qe[:], lhsT=sel[:, t * P:(t + 1) * P],
                                     rhs=qwf[:], start=True, stop=True)

                    # st[e, n] one-hot from cum compares
                    et = sb.tile([P, 1], f32, tag="et")
                    nc.vector.tensor_scalar(out=et[:], in0=piota[:],
                                            scalar1=float(t * P), scalar2=None,
                                            op0=mybir.AluOpType.add)
                    st = sb.tile([P, P], f32, tag="st")
                    nc.vector.tensor_tensor(
                        out=st[:], in0=et[:].to_broadcast([P, P]),
                        in1=cumbc[:, 0:P], op=mybir.AluOpType.is_ge)
                    stlt = sb.tile([P, P], f32, tag="stlt")
                    nc.vector.tensor_tensor(
                        out=stlt[:], in0=et[:].to_broadcast([P, P]),
                        in1=cumbc[:, 1:P + 1], op=mybir.AluOpType.is_lt)
                    nc.vector.tensor_tensor(out=st[:], in0=st[:], in1=stlt[:],
                                            op=mybir.AluOpType.mult)

                    prod = sb.tile([P, HID], f32, tag="prod")
                    nc.vector.tensor_tensor(
                        out=prod[:], in0=kvf[:, :HID], in1=qe[:, :HID],
                        op=mybir.AluOpType.mult)
                    sc = sb.tile([P, HEADS], f32, tag="sc")
                    nc.vector.tensor_reduce(
                        out=sc[:], in_=prod[:].rearrange("p (h d) -> p h d", h=HEADS),
                        axis=mybir.AxisListType.X, op=mybir.AluOpType.add)
                    # apply per-src k scale * per-dst q scale before exp
                    sscp = sb.tile([P, 1], f32, tag="sscp")
                    nc.vector.tensor_tensor(
                        out=sscp[:], in0=ssc[:, 0:1], in1=qe[:, HID:HID + 1],
                        op=mybir.AluOpType.mult)
                    nc.vector.tensor_tensor(
                        out=sc[:], in0=sc[:], in1=sscp[:].to_broadcast([P, HEADS]),
                        op=mybir.AluOpType.mult)
                    nc.scalar.activation(
                        out=sc[:], in_=sc[:],
                        func=mybir.ActivationFunctionType.Exp,
                        scale=1.0 / math.sqrt(HD))
                    msgext = sb.tile([P, HID + HEADS], f32, tag="msgext")
                    nc.vector.tensor_scalar(
                        out=msgext[:, HID:], in0=sc[:],
                        scalar1=CLIP_LO, scalar2=CLIP_HI,
                        op0=mybir.AluOpType.max, op1=mybir.AluOpType.min)
                    # fold the per-src v scale into the message weight
                    sv = sb.tile([P, HEADS], f32, tag="sv")
                    nc.vector.tensor_tensor(
                        out=sv[:], in0=msgext[:, HID:],
                        in1=ssc[:, 1:2].to_broadcast([P, HEADS]),
                        op=mybir.AluOpType.mult)
                    nc.vector.tensor_tensor(
                        out=msgext[:, :HID].rearrange("p (h d) -> p h d", h=HEADS),
                        in0=kvf[:, HID:].rearrange("p (h d) -> p h d", h=HEADS),
                        in1=sv[:][:, :, None].to_broadcast([P, HEADS, HD]),
                        op=mybir.AluOpType.mult)
                    nc.tensor.matmul(out=acc[:], lhsT=st[:], rhs=msgext[:],
                                     start=(t == 0), stop=(t == G - 1))

                zr = sb.tile([P, HEADS], f32, tag="zr")
                nc.vector.tensor_scalar(out=zr[:], in0=acc[:, HID:], scalar1=1e-6,
                                        scalar2=None, op0=mybir.AluOpType.add)
                nc.vector.reciprocal(out=zr[:], in_=zr[:])
                xsb = sb.tile([P, HID], f32, tag="xsb")
                nc.vector.tensor_tensor(
                    out=xsb[:].rearrange("p (h d) -> p h d", h=HEADS),
                    in0=acc[:, :HID].rearrange("p (h d) -> p h d", h=HEADS),
                    in1=zr[:][:, :, None].to_broadcast([P, HEADS, HD]),
                    op=mybir.AluOpType.mult)

                # per-row int8 quantization: scale = absmax/127, guarded vs 0
                xab = sb.tile([P, HID], f32, tag="xab")
                nc.scalar.activation(out=xab[:], in_=xsb[:],
                                     func=mybir.ActivationFunctionType.Abs)
                rmax = sb.tile([P, 1], f32, tag="rmax")
                nc.vector.tensor_reduce(
                    out=rmax[:], in_=xab[:],
                    axis=mybir.AxisListType.X, op=mybir.AluOpType.max)
                nc.vector.tensor_scalar(out=rmax[:], in0=rmax[:], scalar1=1e-30,
                                        scalar2=None, op0=mybir.AluOpType.add)
                rinv = sb.tile([P, 1], f32, tag="rinv")
                nc.vector.reciprocal(out=rinv[:], in_=rmax[:])
                nc.vector.tensor_scalar(out=rinv[:], in0=rinv[:], scalar1=127.0,
                                        scalar2=None, op0=mybir.AluOpType.mult)
                xq8 = sb.tile([P, HID + 2], i8, tag="xq8")
                nc.vector.tensor_tensor(
                    out=xq8[:, :HID], in0=xsb[:], in1=rinv[:].to_broadcast([P, HID]),
                    op=mybir.AluOpType.mult)
                nc.vector.tensor_scalar(
                    out=xq8[:, HID:HID + 2].bitcast(f16), in0=rmax[:],
                    scalar1=1.0 / 127.0, scalar2=None, op0=mybir.AluOpType.mult)

                nc.gpsimd.indirect_dma_start(
                    out=xout[:], out_offset=bass.IndirectOffsetOnAxis(
                        ap=m4[:, 0:1], axis=0),
                    in_=xq8[:], in_offset=None)
    nc.finalize()
    return nc


def _make_runner(nc):
    """Cached PJRT runner: jitted shard_map over 8 cores with device-created
    donated zero output buffers (avoids uploading zeros over the tunnel)."""
    import jax
    import jax.numpy as jnp
    from jax.experimental.shard_map import shard_map
    from jax.sharding import Mesh, PartitionSpec, NamedSharding
    from concourse.bass2jax import (
        _bass_exec_p, install_neuronx_cc_hook, partition_id_tensor)

    install_neuronx_cc_hook()
    partition_name = nc.partition_id_tensor.name if nc.partition_id_tensor else None

    in_names, out_names, out_avals = [], [], []
    for alloc in nc.m.functions[0].allocations:
        if not isinstance(alloc, mybir.MemoryLocationSet):
            continue
        name = alloc.memorylocations[0].name
        if alloc.kind == "ExternalInput":
            if name != partition_name:
                in_names.append(name)
        elif alloc.kind == "ExternalOutput":
            shape = tuple(alloc.tensor_shape)
            dtype = mybir.dt.np(alloc.dtype)
            out_names.append(name)
            out_avals.append(jax.core.ShapedArray(shape, dtype))

    n_params = len(in_names)
    n_outs = len(out_names)
    all_names = list(in_names) + list(out_names)
    if partition_name is not None:
        all_names.append(partition_name)
    donate = tuple(range(n_params, n_params + n_outs))

    def _body(*args):
        operands = list(args)
        if partition_name is not None:
            operands.append(partition_id_tensor())
        outs = _bass_exec_p.bind(
            *operands,
            out_avals=tuple(out_avals),
            in_names=tuple(all_names),
            out_names=tuple(out_names),
            lowering_input_output_aliases=(),
            sim_require_finite=True,
            sim_require_nnan=True,
            nc=nc,
        )
        return tuple(outs)

    devices = jax.devices()[:NCORES]
    mesh = Mesh(np.asarray(devices), ("core",))
    in_specs = (PartitionSpec("core"),) * (n_params + n_outs)
    out_specs = (PartitionSpec("core"),) * n_outs
    sharded = jax.jit(
        shard_map(_body, mesh=mesh, in_specs=in_specs, out_specs=out_specs,
                  check_rep=False),
        donate_argnums=donate, keep_unused=True)

    zspec = NamedSharding(mesh, PartitionSpec("core"))
    zshapes = [(NCORES * a.shape[0], *a.shape[1:]) for a in out_avals]
    zdtypes = [a.dtype for a in out_avals]
    zeros_fn = jax.jit(
        lambda: tuple(jnp.zeros(s, d) for s, d in zip(zshapes, zdtypes)),
        out_shardings=tuple(zspec for _ in out_avals))

    def run(concat_in_map):
        ins = [concat_in_map[name] for name in in_names]
        zeros = _cache.pop("zeros_next", None)
        if zeros is None:
            zeros = zeros_fn()
        outs = sharded(*ins, *zeros)
        return {name: outs[i] for i, name in enumerate(out_names)}

    def prefetch_zeros():
        # donated zero buffers for the next call (device-side, async)
        if "zeros_next" not in _cache:
            _cache["zeros_next"] = zeros_fn()

    return run, zspec, prefetch_zeros


def kernel(q, k, v, edge_index):
    import jax
    q = np.ascontiguousarray(np.asarray(q, np.float32).reshape(N, HID))
    k = np.ascontiguousarray(np.asarray(k, np.float32).reshape(N, HID))
    v = np.ascontiguousarray(np.asarray(v, np.float32).reshape(N, HID))
    e = np.asarray(edge_index)

    pack0 = None
    if "runner" not in _cache:
        pack0 = _pack(e[0], e[1])
        nc = _build(pack0[1])
        _cache["runner"] = (_make_runner(nc), pack0[1])
    (run, zspec, prefetch_zeros), Gmax_built = _cache["runner"]

    # one merged node table [N, 390] -> single sharded device_put (a sharded
    # put costs one ~45ms fixed overhead + bytes/47MBps on the tunnel)
    tab = _cache.get("tab")
    if tab is None:
        tab = _cache["tab"] = np.empty((N, TW), np.int8)
    clib = _get_clib()
    if clib is not None:
        # fused single-read-pass per-row quant straight into the table
        clib.quant_rows(k.ctypes.data, N, HID, tab.ctypes.data, TW, 0, 2 * HID)
        clib.quant_rows(v.ctypes.data, N, HID, tab.ctypes.data, TW, HID,
                        2 * HID + 2)
        clib.quant_rows(q.ctypes.data, N, HID, tab.ctypes.data, TW, KVW,
                        KVW + HID)
    else:
        buf = _cache.get("qbuf")
        if buf is None:
            buf = _cache["qbuf"] = np.empty((N, HID), np.float32)

        def quant_rows(x, dst_lo):
            scale = (np.maximum(x.max(axis=1), -x.min(axis=1))
                     .reshape(N, 1) * (1.0 / 127.0) + 1e-30)
            inv = np.float32(1.0) / scale
            np.multiply(x, inv, out=buf)
            np.rint(buf, out=buf)       # integral floats in [-127, 127]
            tab[:, dst_lo:dst_lo + HID] = buf   # exact f32->int8 cast + store
            return scale

        kscale = quant_rows(k, 0)
        vscale = quant_rows(v, HID)
        tab[:, 2 * HID:2 * HID + 2] = kscale.astype(np.float16).view(np.int8)
        tab[:, 2 * HID + 2:KVW] = vscale.astype(np.float16).view(np.int8)
        qscale = quant_rows(q, KVW)
        tab[:, KVW + HID:] = qscale.astype(np.float16).view(np.int8)
    tab_dev = jax.device_put(tab, zspec)

    # edge packing (overlaps with the async table upload)
    per_core, Gmax = pack0 if pack0 is not None else _pack(e[0], e[1])
    if Gmax != Gmax_built:   # unexpected input distribution: rebuild
        nc = _build(Gmax)
        _cache["runner"] = (_make_runner(nc), Gmax)
        (run, zspec, prefetch_zeros), Gmax_built = _cache["runner"]

    concat = {
        "tab": tab_dev,
        "srcs": np.concatenate([pc["srcs"] for pc in per_core], axis=0),
        "meta4": np.concatenate([pc["meta4"] for pc in per_core], axis=0),
        "cumf": np.concatenate([pc["cumf"] for pc in per_core], axis=0),
    }
    outs = run(concat)
    xo = outs["xout"]
    try:
        xo.copy_to_host_async()
    except Exception:
        pass

    # dequantize shard-by-shard as download bytes arrive
    out = np.empty((N, HID), np.float32)
    rows_per = SH + 1
    for shard in xo.addressable_shards:
        r0 = shard.index[0].start or 0
        core = r0 // rows_per
        xs = np.asarray(shard.data)                      # [SH+1, 130] int8
        blk = out[core * SH:(core + 1) * SH]
        if clib is not None and xs.flags.c_contiguous:
            clib.dequant_rows(xs.ctypes.data, SH, HID, QW, blk.ctypes.data)
        else:
            scale = np.ascontiguousarray(xs[:SH, HID:]).view(np.float16) \
                .astype(np.float32)
            np.multiply(xs[:SH, :HID], scale, out=blk)
    prefetch_zeros()
    return out.reshape(1, N, HID)


# revision 7
# speedup vs baseline: 1.0257x; 1.0257x over previous
"""GNN sparse-attention message passing on 8 Trainium2 NeuronCores (v2).

The axon tunnel (~47MB/s up, ~30MB/s down, shared serial channel) dominates
end-to-end time, so the kernel minimizes host<->device bytes and overlaps
host work with the upload:
- k, v, q ship as ONE int8 table [N, 390] SHARDED (N/8 rows per core):
  row = [k int8 x128 | v int8 x128 | kscale f16 | vscale f16 | q int8 x128 |
  qscale f16], every tensor per-row-scaled. The kv half is all-gathered on
  device over NeuronLink into a full DRAM table; the q half stays local
  (edges are sharded by destination node).
- Edge indices ship as uint16 src ids laid out in tile slots, plus a small
  per-group metadata table (out row, group base, cumulative edge counts).
  The destination one-hot matrices are reconstructed ON DEVICE from the
  cumulative counts (edges are dst-sorted), so no per-edge dst bytes ship.
- Per-edge q rows are NOT gathered row-by-row: each group's dst window is
  <=128 consecutive nodes, so the q window loads with one 128-descriptor
  gather per GROUP and a one-hot matmul (sel[n,e]) routes window rows to
  edge slots. This halves SWDGE descriptor generation (the device-side
  bottleneck, ~150ns/row) vs gathering q per edge.
- Output is quantized on device to int8 with a per-row fp16 scale; all
  scatter trash rows collapse onto one extra row so the download slab is
  [N/8+1, 130] per core. The host prefetches the download and dequantizes
  shard-by-shard as bytes arrive.

Compute (edge parallelism, sharded by destination node):
- Sort edges by dst (uint16 radix argsort); core c owns dst range
  [c*6250, (c+1)*6250).
- Per core, pack edges into groups of G tiles x 128 edges; each group's dst
  nodes lie in a window of <=128 consecutive node ids. cum[n] = number of
  group edges with dst < base+n gives both one-hot masks by comparison
  against an edge-slot iota: st[e,n] = (cum[n] <= e < cum[n+1]).
- Per tile: gather kv rows per edge via indirect DMA, widen int8->f32;
  qe = sel^T @ qwin (one-hot matmul); score = exp(clip(ks*qs*sum k*q / 4));
  msg = v * score * vs; one-hot matmul accumulates [wV | Z] in PSUM.
- Divide, row-quantize, indirect-scatter window rows to the output slab.
"""
import math

import numpy as np

import concourse.bass as bass
import concourse.tile as tile
from concourse import bacc, mybir

N = 50000
E = 800000
HID = 128
HEADS = 8
HD = 16
NCORES = 8
SH = N // NCORES          # nodes per core (fixed bounds)
G = 12                    # tiles per group
P = 128
GP = G * P
KVW = 2 * HID + 4         # kv row: 128 k + 128 v + f16 k-scale + f16 v-scale
QW = HID + 2              # q row: 128 q + f16 q-scale
TW = KVW + QW             # one merged upload row per node
CLIP_LO = float(np.exp(-5.0))
CLIP_HI = float(np.exp(5.0))

_cache = {}

_C_SRC = r"""
#include <stdint.h>
#include <math.h>

static inline uint16_t f32_to_f16(float f) {
    uint32_t x; __builtin_memcpy(&x, &f, 4);
    uint32_t r = x + 0x00001000u;              /* round mantissa (carries ok) */
    uint32_t sign = (r >> 16) & 0x8000u;
    int32_t exp = (int32_t)((r >> 23) & 0xff) - 127 + 15;
    uint32_t man = r & 0x7fffffu;
    if (exp <= 0) return (uint16_t)sign;              /* flush tiny to 0 */
    if (exp >= 31) return (uint16_t)(sign | 0x7bffu); /* clamp to max finite */
    return (uint16_t)(sign | ((uint32_t)exp << 10) | (man >> 13));
}

static inline float f16_to_f32(uint16_t h) {
    uint32_t sign = ((uint32_t)h & 0x8000u) << 16;
    uint32_t exp = (h >> 10) & 0x1fu;
    uint32_t man = h & 0x3ffu;
    uint32_t x;
    if (exp == 0) {
        if (!man) x = sign;
        else {
            int sh = 0;
            while (!(man & 0x400u)) { man <<= 1; sh++; }
            man &= 0x3ffu;
            x = sign | ((uint32_t)(113 - sh) << 23) | (man << 13);
        }
    } else if (exp == 31) {
        x = sign | 0x7f800000u | (man << 13);
    } else {
        x = sign | ((exp - 15 + 127) << 23) | (man << 13);
    }
    float f; __builtin_memcpy(&f, &x, 4);
    return f;
}

/* per-row absmax int8 quant; writes values + f16 scale into the packed table */
void quant_rows(const float* restrict x, int64_t nrows, int64_t w,
                int8_t* restrict tab, int64_t tab_stride,
                int64_t val_off, int64_t scale_off) {
    for (int64_t i = 0; i < nrows; i++) {
        const float* row = x + i * w;
        float m = 0.0f;
        for (int64_t j = 0; j < w; j++) {
            float a = fabsf(row[j]);
            m = a > m ? a : m;
        }
        float scale = m * (1.0f / 127.0f) + 1e-30f;
        float inv = 1.0f / scale;
        int8_t* dst = tab + i * tab_stride + val_off;
        for (int64_t j = 0; j < w; j++) {
            dst[j] = (int8_t)(int32_t)rintf(row[j] * inv);
        }
        uint16_t hb = f32_to_f16(scale);
        __builtin_memcpy(tab + i * tab_stride + scale_off, &hb, 2);
    }
}

/* int8 row * f16 row-scale -> f32 */
void dequant_rows(const int8_t* restrict xs, int64_t nrows, int64_t w,
                  int64_t stride, float* restrict out) {
    for (int64_t i = 0; i < nrows; i++) {
        const int8_t* row = xs + i * stride;
        uint16_t hb; __builtin_memcpy(&hb, row + w, 2);
        float s = f16_to_f32(hb);
        float* o = out + i * w;
        for (int64_t j = 0; j < w; j++) o[j] = (float)row[j] * s;
    }
}
"""


def _get_clib():
    """Compile the streaming quant/dequant helper once; None -> numpy path."""
    if "clib" in _cache:
        return _cache["clib"]
    lib = None
    try:
        import ctypes, subprocess, tempfile, os
        d = tempfile.mkdtemp(prefix="gnnq_")
        src = os.path.join(d, "q.c")
        so = os.path.join(d, "q.so")
        with open(src, "w") as f:
            f.write(_C_SRC)
        for flags in (["-O3", "-march=native", "-ffast-math", "-fno-math-errno"],
                      ["-O2"]):
            try:
                subprocess.run(["cc", *flags, "-shared", "-fPIC", src, "-o", so],
                               check=True, capture_output=True, timeout=60)
                lib = ctypes.CDLL(so)
                lib.quant_rows.argtypes = [
                    ctypes.c_void_p, ctypes.c_int64, ctypes.c_int64,
                    ctypes.c_void_p, ctypes.c_int64, ctypes.c_int64,
                    ctypes.c_int64]
                lib.dequant_rows.argtypes = [
                    ctypes.c_void_p, ctypes.c_int64, ctypes.c_int64,
                    ctypes.c_int64, ctypes.c_void_p]
                break
            except Exception:
                lib = None
    except Exception:
        lib = None
    _cache["clib"] = lib
    return lib


def _pack(e_src, e_dst):
    """Sort edges by dst, shard across fixed core ranges, pack into groups.

    Vectorized: python loops only over cores x groups (~500 iterations).
    Produces per core:
      srcs  [P, Gmax*G] u16 — src node id per edge slot (slot = t*128+p)
      meta4 [P, 4*Gmax] u16 — per group g cols 4g..4g+3 =
            [out row (or SH for trash), group base, cum[p], cum[p+1]]
      cumf  [Gmax, 129] u16 — the same cumulative counts, free-axis layout
    """
    order = np.argsort(e_dst.astype(np.uint16), kind="stable")
    s = e_src.astype(np.uint16)[order]
    d = e_dst.astype(np.int32)[order]
    deg = np.bincount(d, minlength=N)
    cum = np.concatenate([[0], np.cumsum(deg)])  # cum[n] = edges with dst < n

    # greedy group boundaries per core: window <=P nodes, <=G*P edges
    core_groups = []
    for c in range(NCORES):
        n0, n1 = c * SH, (c + 1) * SH
        bases = []
        ni = n0
        while ni < n1:
            bases.append(ni)
            cap_node = min(ni + P, n1)
            cap_edge = int(np.searchsorted(cum, cum[ni] + GP, side="right")) - 1
            ni = max(ni + 1, min(cap_node, cap_edge))
        core_groups.append(bases)
    Gmax = max(len(b) for b in core_groups)

    per_core = []
    r128 = np.arange(P)
    r129 = np.arange(P + 1)
    for c in range(NCORES):
        n0, n1 = c * SH, (c + 1) * SH
        bases = np.asarray(core_groups[c], np.int64)
        ng = len(bases)
        nxt = np.concatenate([bases[1:], [n1]])
        e0s, e1s = cum[bases], cum[nxt]          # edge ranges per group
        ne = e1s - e0s
        assert int(ne.max(initial=0)) <= GP, "group edge overflow"

        # src id for every edge slot of this core in one shot
        srcs = np.zeros((Gmax, GP), np.uint16)
        ce0, ce1 = cum[n0], cum[n1]
        slot = np.repeat(np.arange(ng) * GP - (e0s - ce0), ne) \
            + np.arange(ce1 - ce0)
        srcs.reshape(-1)[slot] = s[ce0:ce1]

        # group-local cumulative edge counts over the 128-node window
        idx = np.minimum(bases[:, None] + r129[None, :], nxt[:, None])
        cumg = (cum[idx] - e0s[:, None]).astype(np.uint16)      # [ng, 129]
        cumf = np.zeros((Gmax, P + 1), np.uint16)
        cumf[:ng] = cumg

        span = np.minimum(nxt - bases, P)                       # [ng]
        rows = (bases[:, None] - n0) + r128[None, :]            # [ng, P]
        m4 = np.zeros((Gmax, 4, P), np.uint16)
        m4[:, 0, :] = SH                                        # trash row
        m4[:ng, 0, :] = np.where(r128[None, :] < span[:, None], rows, SH)
        m4[:ng, 1, :] = (bases[:, None] - n0).astype(np.uint16)
        m4[:ng, 2, :] = cumg[:, :P]
        m4[:ng, 3, :] = cumg[:, 1:]

        per_core.append({
            "srcs": np.ascontiguousarray(
                srcs.reshape(Gmax, G, P).transpose(2, 0, 1)).reshape(P, Gmax * G),
            "meta4": np.ascontiguousarray(
                m4.transpose(2, 0, 1)).reshape(P, 4 * Gmax),
            "cumf": cumf,
        })
    return per_core, Gmax


def _build(Gmax):
    nc = bacc.Bacc(None, target_bir_lowering=False, num_devices=NCORES)
    f32 = mybir.dt.float32
    f16 = mybir.dt.float16
    i32 = mybir.dt.int32
    i8 = mybir.dt.int8
    u16 = mybir.dt.uint16
    tab = nc.declare_dram_parameter("tab", [SH, TW], i8, isOutput=False)
    srcs = nc.declare_dram_parameter("srcs", [P, Gmax * G], u16, isOutput=False)
    meta4 = nc.declare_dram_parameter("meta4", [P, 4 * Gmax], u16, isOutput=False)
    cumf = nc.declare_dram_parameter("cumf", [Gmax, P + 1], u16, isOutput=False)
    xout = nc.declare_dram_parameter("xout", [SH + 1, QW], i8, isOutput=True)

    # bounce buffers for the all-gather (collectives can't touch I/O tensors)
    agkv = nc.dram_tensor("agkv", [SH, KVW], i8)
    kvfull = nc.dram_tensor("kvfull", [N, KVW], i8)
    qfull = nc.dram_tensor("qfull", [SH, QW], i8)

    with tile.TileContext(nc) as tc:
        with tc.tile_pool(name="const", bufs=1) as cp, \
             tc.tile_pool(name="sbuf", bufs=3) as sb, \
             tc.tile_pool(name="meta", bufs=2) as mp, \
             tc.tile_pool(name="psum", bufs=2, space="PSUM") as ps:
            nc.sync.dma_start(out=agkv[:], in_=tab[:, :KVW])
            nc.sync.dma_start(out=qfull[:], in_=tab[:, KVW:])
            nc.gpsimd.collective_compute(
                "AllGather", mybir.AluOpType.bypass,
                replica_groups=[list(range(NCORES))],
                ins=[agkv[:].opt()], outs=[kvfull[:].opt()])

            # constants: edge-slot iota along free axis, partition iota
            eii = cp.tile([P, GP], i32)
            nc.gpsimd.iota(eii[:], pattern=[[1, GP]], base=0, channel_multiplier=0)
            eiota = cp.tile([P, GP], f32)
            nc.vector.tensor_copy(out=eiota[:], in_=eii[:])
            pii = cp.tile([P, 1], i32)
            nc.gpsimd.iota(pii[:], pattern=[[1, 1]], base=0, channel_multiplier=1)
            piota = cp.tile([P, 1], f32)
            nc.vector.tensor_copy(out=piota[:], in_=pii[:])
            ones1 = cp.tile([1, P], f32)
            nc.vector.memset(ones1[:], 1.0)

            for g in range(Gmax):
                srcs_sb = mp.tile([P, G], i32, tag="srcs")
                nc.gpsimd.dma_start(out=srcs_sb[:], in_=srcs[:, g * G:(g + 1) * G])
                m4 = mp.tile([P, 4], i32, tag="m4")
                nc.gpsimd.dma_start(out=m4[:], in_=meta4[:, 4 * g:4 * g + 4])
                cumf_i = mp.tile([1, P + 1], i32, tag="cumfi")
                nc.gpsimd.dma_start(out=cumf_i[:], in_=cumf[g:g + 1, :])
                cumf_f = mp.tile([1, P + 1], f32, tag="cumff")
                nc.vector.tensor_copy(out=cumf_f[:], in_=cumf_i[:])

                # broadcast cum over partitions: [1,129] -> PSUM [128,129]
                cumbc = ps.tile([P, P + 1], f32, space="PSUM", tag="cumbc")
                nc.tensor.matmul(out=cumbc[:], lhsT=ones1[:], rhs=cumf_f[:],
                                 start=True, stop=True)

                # q window rows: qoff = min(gbase + p, SH-1)
                qoff = mp.tile([P, 1], i32, tag="qoff")
                nc.vector.tensor_tensor(out=qoff[:], in0=m4[:, 1:2], in1=pii[:],
                                        op=mybir.AluOpType.add)
                nc.vector.tensor_scalar(out=qoff[:], in0=qoff[:], scalar1=SH - 1,
                                        scalar2=None, op0=mybir.AluOpType.min)
                qt8 = mp.tile([P, QW], i8, tag="qt8")
                nc.gpsimd.indirect_dma_start(
                    out=qt8[:], out_offset=None, in_=qfull[:],
                    in_offset=bass.IndirectOffsetOnAxis(ap=qoff[:, 0:1], axis=0))
                qwf = mp.tile([P, HID + 1], f32, tag="qwf")
                nc.vector.tensor_copy(out=qwf[:, :HID], in_=qt8[:, :HID])
                nc.vector.tensor_copy(
                    out=qwf[:, HID:HID + 1],
                    in_=qt8[:, HID:HID + 2].bitcast(f16))

                # sel[n, e] = (cum[n] <= e) & (e < cum[n+1]) over the whole group
                cumlh = mp.tile([P, 2], f32, tag="cumlh")
                nc.vector.tensor_copy(out=cumlh[:], in_=m4[:, 2:4])
                sel = mp.tile([P, GP], f32, tag="sel")
                nc.vector.tensor_tensor(
                    out=sel[:], in0=eiota[:],
                    in1=cumlh[:, 0:1].to_broadcast([P, GP]),
                    op=mybir.AluOpType.is_ge)
                sellt = mp.tile([P, GP], f32, tag="sellt")
                nc.vector.tensor_tensor(
                    out=sellt[:], in0=eiota[:],
                    in1=cumlh[:, 1:2].to_broadcast([P, GP]),
                    op=mybir.AluOpType.is_lt)
                nc.vector.tensor_tensor(out=sel[:], in0=sel[:], in1=sellt[:],
                                        op=mybir.AluOpType.mult)

                acc = ps.tile([P, HID + HEADS], f32, space="PSUM", tag="acc")
                for t in range(G):
                    kvt8 = sb.tile([P, KVW], i8, tag="kvt8")
                    nc.gpsimd.indirect_dma_start(
                        out=kvt8[:], out_offset=None, in_=kvfull[:],
                        in_offset=bass.IndirectOffsetOnAxis(
                            ap=srcs_sb[:, t:t + 1], axis=0))
                    kvf = sb.tile([P, 2 * HID], f32, tag="kvf")
                    nc.vector.tensor_copy(out=kvf[:], in_=kvt8[:, :2 * HID])
                    ssc = sb.tile([P, 2], f32, tag="ssc")
                    nc.vector.tensor_copy(
                        out=ssc[:, 0:1],
                        in_=kvt8[:, 2 * HID:2 * HID + 2].bitcast(f16))
                    nc.vector.tensor_copy(
                        out=ssc[:, 1:2],
                        in_=kvt8[:, 2 * HID + 2:2 * HID + 4].bitcast(f16))

                    # per-edge q row via one-hot matmul
                    qe = ps.tile([P, HID + 1], f32, space="PSUM", tag="qe")
                    nc.tensor.matmul(out=qe[:], lhsT=sel[:, t * P:(t + 1) * P],
                                     rhs=qwf[:], start=True, stop=True)

                    # st[e, n] one-hot from cum compares
                    et = sb.tile([P, 1], f32, tag="et")
                    nc.vector.tensor_scalar(out=et[:], in0=piota[:],
                                            scalar1=float(t * P), scalar2=None,
                                            op0=mybir.AluOpType.add)
                    st = sb.tile([P, P], f32, tag="st")
                    nc.vector.tensor_tensor(
                        out=st[:], in0=et[:].to_broadcast([P, P]),
                        in1=cumbc[:, 0:P], op=mybir.AluOpType.is_ge)
                    stlt = sb.tile([P, P], f32, tag="stlt")
                    nc.vector.tensor_tensor(
                        out=stlt[:], in0=et[:].to_broadcast([P, P]),
                        in1=cumbc[:, 1:P + 1], op=mybir.AluOpType.is_lt)
                    nc.vector.tensor_tensor(out=st[:], in0=st[:], in1=stlt[:],
                                            op=mybir.AluOpType.mult)

                    prod = sb.tile([P, HID], f32, tag="prod")
                    nc.vector.tensor_tensor(
                        out=prod[:], in0=kvf[:, :HID], in1=qe[:, :HID],
                        op=mybir.AluOpType.mult)
                    sc = sb.tile([P, HEADS], f32, tag="sc")
                    nc.vector.tensor_reduce(
                        out=sc[:], in_=prod[:].rearrange("p (h d) -> p h d", h=HEADS),
                        axis=mybir.AxisListType.X, op=mybir.AluOpType.add)
                    # apply per-src k scale * per-dst q scale before exp
                    sscp = sb.tile([P, 1], f32, tag="sscp")
                    nc.vector.tensor_tensor(
                        out=sscp[:], in0=ssc[:, 0:1], in1=qe[:, HID:HID + 1],
                        op=mybir.AluOpType.mult)
                    nc.vector.tensor_tensor(
                        out=sc[:], in0=sc[:], in1=sscp[:].to_broadcast([P, HEADS]),
                        op=mybir.AluOpType.mult)
                    nc.scalar.activation(
                        out=sc[:], in_=sc[:],
                        func=mybir.ActivationFunctionType.Exp,
                        scale=1.0 / math.sqrt(HD))
                    msgext = sb.tile([P, HID + HEADS], f32, tag="msgext")
                    nc.vector.tensor_scalar(
                        out=msgext[:, HID:], in0=sc[:],
                        scalar1=CLIP_LO, scalar2=CLIP_HI,
                        op0=mybir.AluOpType.max, op1=mybir.AluOpType.min)
                    # fold the per-src v scale into the message weight
                    sv = sb.tile([P, HEADS], f32, tag="sv")
                    nc.vector.tensor_tensor(
                        out=sv[:], in0=msgext[:, HID:],
                        in1=ssc[:, 1:2].to_broadcast([P, HEADS]),
                        op=mybir.AluOpType.mult)
                    nc.vector.tensor_tensor(
                        out=msgext[:, :HID].rearrange("p (h d) -> p h d", h=HEADS),
                        in0=kvf[:, HID:].rearrange("p (h d) -> p h d", h=HEADS),
                        in1=sv[:][:, :, None].to_broadcast([P, HEADS, HD]),
                        op=mybir.AluOpType.mult)
                    nc.tensor.matmul(out=acc[:], lhsT=st[:], rhs=msgext[:],
                                     start=(t == 0), stop=(t == G - 1))

                zr = sb.tile([P, HEADS], f32, tag="zr")
                nc.vector.tensor_scalar(out=zr[:], in0=acc[:, HID:], scalar1=1e-6,
                                        scalar2=None, op0=mybir.AluOpType.add)
                nc.vector.reciprocal(out=zr[:], in_=zr[:])
                xsb = sb.tile([P, HID], f32, tag="xsb")
                nc.vector.tensor_tensor(
                    out=xsb[:].rearrange("p (h d) -> p h d", h=HEADS),
                    in0=acc[:, :HID].rearrange("p (h d) -> p h d", h=HEADS),
                    in1=zr[:][:, :, None].to_broadcast([P, HEADS, HD]),
                    op=mybir.AluOpType.mult)

                # per-row int8 quantization: scale = absmax/127, guarded vs 0
                xab = sb.tile([P, HID], f32, tag="xab")
                nc.scalar.activation(out=xab[:], in_=xsb[:],
                                     func=mybir.ActivationFunctionType.Abs)
                rmax = sb.tile([P, 1], f32, tag="rmax")
                nc.vector.tensor_reduce(
                    out=rmax[:], in_=xab[:],
                    axis=mybir.AxisListType.X, op=mybir.AluOpType.max)
                nc.vector.tensor_scalar(out=rmax[:], in0=rmax[:], scalar1=1e-30,
                                        scalar2=None, op0=mybir.AluOpType.add)
                rinv = sb.tile([P, 1], f32, tag="rinv")
                nc.vector.reciprocal(out=rinv[:], in_=rmax[:])
                nc.vector.tensor_scalar(out=rinv[:], in0=rinv[:], scalar1=127.0,
                                        scalar2=None, op0=mybir.AluOpType.mult)
                xq8 = sb.tile([P, HID + 2], i8, tag="xq8")
                nc.vector.tensor_tensor(
                    out=xq8[:, :HID], in0=xsb[:], in1=rinv[:].to_broadcast([P, HID]),
                    op=mybir.AluOpType.mult)
                nc.vector.tensor_scalar(
                    out=xq8[:, HID:HID + 2].bitcast(f16), in0=rmax[:],
                    scalar1=1.0 / 127.0, scalar2=None, op0=mybir.AluOpType.mult)

                nc.gpsimd.indirect_dma_start(
                    out=xout[:], out_offset=bass.IndirectOffsetOnAxis(
                        ap=m4[:, 0:1], axis=0),
                    in_=xq8[:], in_offset=None)
    nc.finalize()
    return nc


def _make_runner(nc):
    """Cached PJRT runner: jitted shard_map over 8 cores with device-created
    donated zero output buffers (avoids uploading zeros over the tunnel)."""
    import jax
    import jax.numpy as jnp
    from jax.experimental.shard_map import shard_map
    from jax.sharding import Mesh, PartitionSpec, NamedSharding
    from concourse.bass2jax import (
        _bass_exec_p, install_neuronx_cc_hook, partition_id_tensor)

    install_neuronx_cc_hook()
    partition_name = nc.partition_id_tensor.name if nc.partition_id_tensor else None

    in_names, out_names, out_avals = [], [], []
    for alloc in nc.m.functions[0].allocations:
        if not isinstance(alloc, mybir.MemoryLocationSet):
            continue
        name = alloc.memorylocations[0].name
        if alloc.kind == "ExternalInput":
            if name != partition_name:
                in_names.append(name)
        elif alloc.kind == "ExternalOutput":
            shape = tuple(alloc.tensor_shape)
            dtype = mybir.dt.np(alloc.dtype)
            out_names.append(name)
            out_avals.append(jax.core.ShapedArray(shape, dtype))

    n_params = len(in_names)
    n_outs = len(out_names)
    all_names = list(in_names) + list(out_names)
    if partition_name is not None:
        all_names.append(partition_name)
    donate = tuple(range(n_params, n_params + n_outs))

    def _body(*args):
        operands = list(args)
        if partition_name is not None:
            operands.append(partition_id_tensor())
        outs = _bass_exec_p.bind(
            *operands,
            out_avals=tuple(out_avals),
            in_names=tuple(all_names),
            out_names=tuple(out_names),
            lowering_input_output_aliases=(),
            sim_require_finite=True,
            sim_require_nnan=True,
            nc=nc,
        )
        return tuple(outs)

    devices = jax.devices()[:NCORES]
    mesh = Mesh(np.asarray(devices), ("core",))
    in_specs = (PartitionSpec("core"),) * (n_params + n_outs)
    out_specs = (PartitionSpec("core"),) * n_outs
    sharded = jax.jit(
        shard_map(_body, mesh=mesh, in_specs=in_specs, out_specs=out_specs,
                  check_rep=False),
        donate_argnums=donate, keep_unused=True)

    zspec = NamedSharding(mesh, PartitionSpec("core"))
    zshapes = [(NCORES * a.shape[0], *a.shape[1:]) for a in out_avals]
    zdtypes = [a.dtype for a in out_avals]
    zeros_fn = jax.jit(
        lambda: tuple(jnp.zeros(s, d) for s, d in zip(zshapes, zdtypes)),
        out_shardings=tuple(zspec for _ in out_avals))

    def run(concat_in_map):
        ins = [concat_in_map[name] for name in in_names]
        zeros = _cache.pop("zeros_next", None)
        if zeros is None:
            zeros = zeros_fn()
        outs = sharded(*ins, *zeros)
        return {name: outs[i] for i, name in enumerate(out_names)}

    def prefetch_zeros():
        # donated zero buffers for the next call (device-side, async)
        if "zeros_next" not in _cache:
            _cache["zeros_next"] = zeros_fn()

    return run, zspec, prefetch_zeros


def kernel(q, k, v, edge_index):
    import jax
    q = np.ascontiguousarray(np.asarray(q, np.float32).reshape(N, HID))
    k = np.ascontiguousarray(np.asarray(k, np.float32).reshape(N, HID))
    v = np.ascontiguousarray(np.asarray(v, np.float32).reshape(N, HID))
    e = np.asarray(edge_index)

    pack0 = None
    if "runner" not in _cache:
        pack0 = _pack(e[0], e[1])
        nc = _build(pack0[1])
        _cache["runner"] = (_make_runner(nc), pack0[1])
    (run, zspec, prefetch_zeros), Gmax_built = _cache["runner"]

    # one merged node table [N, 390] -> single sharded device_put (a sharded
    # put costs one ~45ms fixed overhead + bytes/47MBps on the tunnel)
    tab = _cache.get("tab")
    if tab is None:
        tab = _cache["tab"] = np.empty((N, TW), np.int8)
    clib = _get_clib()
    if clib is not None:
        # fused single-read-pass per-row quant straight into the table
        clib.quant_rows(k.ctypes.data, N, HID, tab.ctypes.data, TW, 0, 2 * HID)
        clib.quant_rows(v.ctypes.data, N, HID, tab.ctypes.data, TW, HID,
                        2 * HID + 2)
        clib.quant_rows(q.ctypes.data, N, HID, tab.ctypes.data, TW, KVW,
                        KVW + HID)
    else:
        buf = _cache.get("qbuf")
        if buf is None:
            buf = _cache["qbuf"] = np.empty((N, HID), np.float32)

        def quant_rows(x, dst_lo):
            scale = (np.maximum(x.max(axis=1), -x.min(axis=1))
                     .reshape(N, 1) * (1.0 / 127.0) + 1e-30)
            inv = np.float32(1.0) / scale
            np.multiply(x, inv, out=buf)
            np.rint(buf, out=buf)       # integral floats in [-127, 127]
            tab[:, dst_lo:dst_lo + HID] = buf   # exact f32->int8 cast + store
            return scale

        kscale = quant_rows(k, 0)
        vscale = quant_rows(v, HID)
        tab[:, 2 * HID:2 * HID + 2] = kscale.astype(np.float16).view(np.int8)
        tab[:, 2 * HID + 2:KVW] = vscale.astype(np.float16).view(np.int8)
        qscale = quant_rows(q, KVW)
        tab[:, KVW + HID:] = qscale.astype(np.float16).view(np.int8)
    tab_dev = jax.device_put(tab, zspec)

    # edge packing (overlaps with the async table upload)
    per_core, Gmax = pack0 if pack0 is not None else _pack(e[0], e[1])
    if Gmax != Gmax_built:   # unexpected input distribution: rebuild
        nc = _build(Gmax)
        _cache["runner"] = (_make_runner(nc), Gmax)
        (run, zspec, prefetch_zeros), Gmax_built = _cache["runner"]

    concat = {
        "tab": tab_dev,
        "srcs": np.concatenate([pc["srcs"] for pc in per_core], axis=0),
        "meta4": np.concatenate([pc["meta4"] for pc in per_core], axis=0),
        "cumf": np.concatenate([pc["cumf"] for pc in per_core], axis=0),
    }
    outs = run(concat)
    xo = outs["xout"]
    try:
        xo.copy_to_host_async()
    except Exception:
        pass

    # dequantize shard-by-shard as download bytes arrive
    out = np.empty((N, HID), np.float32)
    rows_per = SH + 1
    for shard in xo.addressable_shards:
        r0 = shard.index[0].start or 0
        core = r0 // rows_per
        xs = np.asarray(shard.data)                      # [SH+1, 130] int8
        blk = out[core * SH:(core + 1) * SH]
        if clib is not None and xs.flags.c_contiguous:
            clib.dequant_rows(xs.ctypes.data, SH, HID, QW, blk.ctypes.data)
        else:
            scale = np.ascontiguousarray(xs[:SH, HID:]).view(np.float16) \
                .astype(np.float32)
            np.multiply(xs[:SH, :HID], scale, out=blk)
    prefetch_zeros()
    return out.reshape(1, N, HID)
